# revision 19
# baseline (speedup 1.0000x reference)
"""Trainium2 Bass kernel for nn_ClassificationModel (CNN window encoder +
4-layer post-norm transformer + mean-pool classifier head).

Data parallel across 8 NeuronCores: batch N=64 -> 8 samples/core.
"""

import math
import sys

sys.path.insert(0, "/opt/trn_rl_repo")

import numpy as np
import ml_dtypes

import concourse.bass as bass
import concourse.mybir as mybir
import concourse.tile as tile
from concourse import bacc
from concourse.bass import AP
from concourse.bass_utils import run_bass_kernel_spmd

BF = ml_dtypes.bfloat16
F32 = mybir.dt.float32
BF16 = mybir.dt.bfloat16
AX = mybir.AxisListType
OP = mybir.AluOpType
AF = mybir.ActivationFunctionType

# model dims
N, L, W = 64, 128, 256
D, H, NL, DFF = 384, 8, 4, 1536
E = D // H  # 48
CH = [1, 4, 16, 64]
K = 7
NCORES = 8
RPC = N // NCORES          # samples per core = 8
R = RPC * L                # rows per core = 1024
TEMP = 1.0 / math.sqrt(E)
EPS = 1e-5

# conv block sizes (output positions per Toeplitz block)
B0, B1, B2 = 32, 8, 2
NB0, NB1, NB2 = 256 // B0, 128 // B1, 64 // B2  # 8, 16, 32

# constf column layout: idn_f | clsw | ebpe | b0e b1e b2e onesL epsc
CF_IDN, CF_CLSW, CF_EBPE = 0, 128, 131
CF_B0, CF_B1, CF_B2, CF_ONES, CF_EPS = 515, 516, 517, 518, 519
CF_COLS = 520
# constb column layout: idn_b | neg_eb | colsum masks (8 heads x 8 cols)
CB_IDN, CB_NEB, CB_MASK = 0, 128, 512
CB_COLS = CB_MASK + 64


# ---------------------------------------------------------------------------
# host-side weight preparation
# ---------------------------------------------------------------------------

def _pe_np(l, d):
    pos = np.arange(l)[:, None].astype(np.float32)
    i = np.arange(d // 2)[None, :].astype(np.float32)
    ang = pos / np.power(10000.0, 2.0 * i / d)
    pe = np.zeros((l, d), np.float32)
    pe[:, 0::2] = np.sin(ang)
    pe[:, 1::2] = np.cos(ang)
    return pe


# conv source-block overlap enumeration (shared host/device) -----------------

# (Bout, src_size, nsrc, nch): conv0 reads xT tiles (128 pos each);
# conv1 reads pooled0 blocks (16 pos, 4 ch); conv2 reads pooled1 (4 pos, 16 ch)
CONV_GEOM = {
    0: (B0, 128, 2, 1),
    1: (B1, 16, NB0, 4),
    2: (B2, 4, NB1, 16),
}


def overlaps(conv, b):
    """source tiles overlapping output block b's input window; (src, delta)."""
    Bout, src_size, nsrc, _ = CONV_GEOM[conv]
    w0, w1 = Bout * b - 3, Bout * b + Bout + 3
    res = []
    for s in range(nsrc):
        lo, hi = s * src_size, (s + 1) * src_size
        if max(w0, lo) < min(w1, hi):
            res.append((s, lo - Bout * b))
    return res


def conv_deltas(conv):
    Bout = CONV_GEOM[conv][0]
    nb = {0: NB0, 1: NB1, 2: NB2}[conv]
    ds = sorted({d for b in range(nb) for _, d in overlaps(conv, b)})
    return ds


def _m_layout(conv, h, co):
    if conv == 0:
        return (h & 1) * 64 + (h >> 1) * 4 + co
    if conv == 1:
        return (h & 1) * 64 + (h >> 1) * 16 + co
    return h * 64 + co


def _toeplitz_variants(conv, w):
    """w: (C_out, C_in, K). returns (nvar, src_size*nch, 128) f32."""
    Bout, src_size, _, nch = CONV_GEOM[conv]
    cout = w.shape[0]
    ds = conv_deltas(conv)
    T = np.zeros((len(ds), src_size * nch, 128), np.float32)
    for vi, delta in enumerate(ds):
        for hp in range(src_size):
            for h in range(Bout):
                k = delta + hp - h + 3
                if 0 <= k < K:
                    for co in range(cout):
                        for ci in range(nch):
                            T[vi, hp * nch + ci, _m_layout(conv, h, co)] = w[co, ci, k]
    return T


def host_prep(inp):
    d = {}
    f32 = np.float32
    d["T0"] = _toeplitz_variants(0, np.asarray(inp["conv_w0"], f32)).astype(BF)
    d["T1"] = _toeplitz_variants(1, np.asarray(inp["conv_w1"], f32)).astype(BF)
    d["T2"] = _toeplitz_variants(2, np.asarray(inp["conv_w2"], f32)).astype(BF)
    b0, b1c, b2c = (np.asarray(inp[f"conv_b{i}"], f32) for i in range(3))
    p = np.arange(128)

    # embed: We_r[c, p, :] = embed_w[(p%64)*32 + 2c + p//64, :]
    ew = np.asarray(inp["embed_w"], f32)  # (2048, 384)
    We_r = np.zeros((16, 128, D), f32)
    for c in range(16):
        for pi in range(128):
            We_r[c, pi] = ew[(pi % 64) * 32 + 2 * c + pi // 64]
    d["We_r"] = We_r.astype(BF)

    eb = np.asarray(inp["embed_b"], f32)
    pe = _pe_np(L, D)
    constf = np.zeros((128, CF_COLS), f32)
    constf[:, CF_IDN:CF_IDN + 128] = np.eye(128, dtype=f32)
    constf[:, CF_CLSW:CF_CLSW + 3] = np.asarray(inp["cls_w"], f32).reshape(3, 128).T
    constf[:, CF_EBPE:CF_EBPE + D] = pe + eb[None, :]
    constf[:, CF_B0] = b0[p % 4]
    constf[:, CF_B1] = b1c[p % 16]
    constf[:, CF_B2] = b2c[p % 64]
    constf[:, CF_ONES] = 1.0 / L
    constf[:, CF_EPS] = EPS
    d["constf"] = constf
    constb = np.zeros((128, CB_COLS), f32)
    constb[:, CB_IDN:CB_IDN + 128] = np.eye(128, dtype=f32)
    constb[:, CB_NEB:CB_NEB + D] = -eb[None, :]
    for h in range(8):
        constb[:, CB_MASK + h * 8 + h] = 1.0
    d["constb"] = constb.astype(BF)

    # Q/K head-pair packing: pair p occupies one 128-col chunk, head 2p at
    # col/partition base 0, head 2p+1 at base 64 (tile_position (64,0)
    # matmuls are fine on this device).
    for nm in ("Wq", "Wk"):
        wsrc = np.asarray(inp[nm], f32)  # (4, 384, 384)
        wpad = np.zeros((NL, D, 128 * (H // 2)), f32)
        for h in range(H):
            base = 128 * (h // 2) + 64 * (h & 1)
            wpad[:, :, base:base + 48] = wsrc[:, :, 48 * h:48 * h + 48]
        d[nm] = wpad.astype(BF)
    for nm in ("Wv", "Wo"):
        d[nm] = np.asarray(inp[nm], f32).astype(BF)  # (4, 384, 384)
    d["W1"] = np.asarray(inp["W1"], f32).astype(BF)  # (4, 384, 1536)
    d["W2"] = np.asarray(inp["W2"], f32).astype(BF)  # (4, 1536, 384)

    def _qk_bias(b):  # (4, 384) -> (4, 128, 4) pair-packed
        out = np.zeros((NL, 128, H // 2), f32)
        for h in range(H):
            base = 64 * (h & 1)
            out[:, base:base + 48, h // 2] = b[:, 48 * h:48 * h + 48]
        return out
    # lsmall: bqp(4) | bkp(4) | b1r(12) -> (4, 128, 20) f32
    b1f = np.asarray(inp["b1"], f32)  # (4, 1536)
    lsmall = np.zeros((NL, 128, 20), f32)
    lsmall[:, :, 0:4] = _qk_bias(np.asarray(inp["bq"], f32))
    lsmall[:, :, 4:8] = _qk_bias(np.asarray(inp["bk"], f32))
    lsmall[:, :, 8:20] = np.stack([b1f[l].reshape(12, 128).T for l in range(NL)])
    d["lsmall"] = lsmall

    # lb7: bv | bo | b2f | g1 | be1 | g2 | be2 -> (4, 128, 7*D) bf16
    lb7 = np.zeros((NL, 128, 7 * D), f32)
    for i, src in enumerate(("bv", "bo", "b2", "g1", "be1", "g2", "be2")):
        a = np.asarray(inp[src], f32)  # (4, 384)
        lb7[:, :, i * D:(i + 1) * D] = np.broadcast_to(a[:, None, :], (NL, 128, D))
    d["lb7"] = lb7.astype(BF)

    d["clsb"] = np.asarray(inp["cls_b"], f32).reshape(1, 1)
    return d


# ---------------------------------------------------------------------------
# device program
# ---------------------------------------------------------------------------

PHASE_MARKS = []

LB = {n: i * D for i, n in enumerate(("bv", "bo", "b2f", "g1", "be1", "g2", "be2"))}


def _mark(nc, label):
    nm = nc.get_next_instruction_name()  # consumes one name; harmless
    PHASE_MARKS.append((label, int(nm.split("-")[-1])))


def build_program(debug=None, do_compile=True, n_layers=NL, phase=99, loop_k=1):
    PHASE_MARKS.clear()
    nc = bacc.Bacc("TRN2", target_bir_lowering=False, debug=False)

    di = {}  # dram inputs
    def dram_in(name, shape, dt=BF16):
        di[name] = nc.dram_tensor(name, list(shape), dt, kind="ExternalInput")
        return di[name]

    x_d = dram_in("xc", (R, W), F32)
    nv0, nv1, nv2 = len(conv_deltas(0)), len(conv_deltas(1)), len(conv_deltas(2))
    T0_d = dram_in("T0", (nv0, 128, 128))
    T1_d = dram_in("T1", (nv1, 64, 128))
    T2_d = dram_in("T2", (nv2, 64, 128))
    We_d = dram_in("We_r", (16, 128, D))
    constf_d = dram_in("constf", (128, CF_COLS), F32)
    constb_d = dram_in("constb", (128, CB_COLS))
    wq_d = dram_in("Wq", (NL, D, 128 * (H // 2)))
    wk_d = dram_in("Wk", (NL, D, 128 * (H // 2)))
    wv_d = dram_in("Wv", (NL, D, D))
    wo_d = dram_in("Wo", (NL, D, D))
    w1_d = dram_in("W1", (NL, D, DFF))
    w2_d = dram_in("W2", (NL, DFF, D))
    lsmall_d = dram_in("lsmall", (NL, 128, 20), F32)
    lb7_d = dram_in("lb7", (NL, 128, 7 * D))
    clsb_d = dram_in("clsb", (1, 1), F32)

    y_d = nc.dram_tensor("yc", [RPC, 1], F32, kind="ExternalOutput")
    dbg_d = None
    if debug is not None:
        dbg_d = nc.dram_tensor("dbg", [R, D], F32, kind="ExternalOutput")

    from contextlib import ExitStack
    with tile.TileContext(nc) as tc, ExitStack() as ctx:
        const = ctx.enter_context(tc.tile_pool(name="const", bufs=1))
        state = ctx.enter_context(tc.tile_pool(name="state", bufs=1))
        wpool = ctx.enter_context(tc.tile_pool(name="wpool", bufs=2))
        work = ctx.enter_context(tc.tile_pool(name="work", bufs=2))
        psum = ctx.enter_context(tc.tile_pool(name="psum", bufs=2, space="PSUM"))

        # --- consts (one DMA each) ---
        T0v = const.tile([128, nv0, 128], BF16, tag="T0v", name="T0v")
        nc.sync.dma_start(T0v[:], T0_d[:].rearrange("v p m -> p v m"))
        T1v = const.tile([64, nv1, 128], BF16, tag="T1v", name="T1v")
        nc.sync.dma_start(T1v[:], T1_d[:].rearrange("v p m -> p v m"))
        T2v = const.tile([64, nv2, 128], BF16, tag="T2v", name="T2v")
        nc.sync.dma_start(T2v[:], T2_d[:].rearrange("v p m -> p v m"))
        We = const.tile([128, 16, D], BF16, tag="We", name="We")
        nc.sync.dma_start(We[:], We_d[:].rearrange("c p m -> p c m"))
        cf = const.tile([128, CF_COLS], F32, tag="cf", name="cf")
        nc.sync.dma_start(cf[:], constf_d[:])
        cb = const.tile([128, CB_COLS], BF16, tag="cb", name="cb")
        nc.sync.dma_start(cb[:], constb_d[:])
        clsb = const.tile([1, 1], F32, tag="clsb", name="clsb")
        nc.sync.dma_start(clsb[:], clsb_d[:])

        idn_f = cf[:, CF_IDN:CF_IDN + 128]
        idn_b = cb[:, CB_IDN:CB_IDN + 128]
        d2i = [{d: i for i, d in enumerate(conv_deltas(c))} for c in range(3)]

        # optional hardware loop around the whole body: lets the timing
        # harness measure marginal per-iteration HW time with the fixed
        # per-exec dispatch overhead amortized away.
        if loop_k > 1:
            ctx.enter_context(tc.For_i(0, loop_k, 1))

        # persistent state
        t_rm = [state.tile([128, D], F32, tag=f"t_rm{rt}", name=f"t_rm{rt}") for rt in range(RPC)]
        t_fm = [state.tile([128, R], BF16, tag=f"t_fm{c}", name=f"t_fm{c}") for c in range(3)]
        o_fm = [state.tile([128, R], BF16, tag=f"o_fm{c}", name=f"o_fm{c}") for c in range(3)]
        h1 = [state.tile([128, R], BF16, tag=f"h1_{c}", name=f"h1_{c}") for c in range(12)]

        # ------------------------------------------------------- CNN + embed
        _mark(nc, 'cnn')

        def conv_pool(ps, n_blk, pooled_out, bias_col, conv):
            """pooled_out[64, n_blk, 128] = relu(max(lo, hi) + bias).

            ps: [128, n_blk*128] psum, even-parity rows 0:64, odd 64:128.
            relu(max(lo,hi)+b) == max(lo+b, relu(hi+b)) so one Act op on the
            upper half plus one DVE scalar_tensor_tensor does it all with no
            partition-shuffle DMA.
            """
            hi_rb = work.tile([64, 512], BF16, tag=f"hi{conv}", name=f"hi{conv}")
            nc.scalar.activation(hi_rb[:, 0:n_blk * 128], ps[64:128, :],
                                 AF.Relu, bias=cf[0:64, bias_col:bias_col + 1])
            nc.vector.scalar_tensor_tensor(
                pooled_out,
                in0=ps[0:64, :].rearrange("p (b r) -> p b r", b=n_blk),
                scalar=cf[0:64, bias_col:bias_col + 1],
                in1=hi_rb[:, 0:n_blk * 128].rearrange("p (b r) -> p b r", b=n_blk),
                op0=OP.add, op1=OP.max)

        for rt in range(RPC):
            x_t = work.tile([128, W], F32, tag="x_t", name="x_t")
            nc.sync.dma_start(x_t[:], x_d[rt * 128:(rt + 1) * 128, :])

            xT = []
            for half in range(2):
                ps = psum.tile([128, 128], F32, tag="psC", name="psC")
                nc.tensor.transpose(ps[:], x_t[:, half * 128:(half + 1) * 128], idn_f)
                xt = work.tile([128, 128], BF16, tag=f"xT{half}", name=f"xT{half}")
                nc.scalar.copy(xt[:], ps[:])
                xT.append(xt)

            # conv0 -> pooled0 (64 = hp*4+co, 8 blocks, 128 rows)
            pooled0 = work.tile([64, NB0, 128], BF16, tag="pooled0", name="pooled0")
            for g in range(2):  # groups of 4 blocks share one psum bank
                ps = psum.tile([128, 512], F32, tag="psA" if g % 2 == 0 else "psD",
                               name="psA")
                for bb in range(4):
                    b = g * 4 + bb
                    ovl = overlaps(0, b)
                    for i, (s, dlt) in enumerate(ovl):
                        nc.tensor.matmul(
                            ps[:, bb * 128:(bb + 1) * 128],
                            lhsT=T0v[:, d2i[0][dlt], :], rhs=xT[s][:],
                            start=(i == 0), stop=(i == len(ovl) - 1))
                conv_pool(ps, 4, pooled0[:, g * 4:(g + 1) * 4, :], CF_B0, 0)

            # conv1 -> pooled1 (64 = hp*16+co, 16 blocks, 128 rows)
            pooled1 = work.tile([64, NB1, 128], BF16, tag="pooled1", name="pooled1")
            for g in range(4):
                ps = psum.tile([128, 512], F32, tag="psA" if g % 2 == 0 else "psD",
                               name="psA")
                for bb in range(4):
                    b = g * 4 + bb
                    ovl = overlaps(1, b)
                    for i, (s, dlt) in enumerate(ovl):
                        nc.tensor.matmul(
                            ps[:, bb * 128:(bb + 1) * 128],
                            lhsT=T1v[:, d2i[1][dlt], :], rhs=pooled0[:, s, :],
                            start=(i == 0), stop=(i == len(ovl) - 1))
                conv_pool(ps, 4, pooled1[:, g * 4:(g + 1) * 4, :], CF_B1, 1)

            # conv2 -> act3 (128 = (b&1)*64+co, 16 chunks, 128 rows)
            act3 = work.tile([128, 16, 128], BF16, tag="act3", name="act3")
            for g in range(8):
                ps = psum.tile([128, 512], F32, tag="psA" if g % 2 == 0 else "psD",
                               name="psA")
                for bb in range(4):
                    b = g * 4 + bb
                    ovl = overlaps(2, b)
                    for i, (s, dlt) in enumerate(ovl):
                        nc.tensor.matmul(
                            ps[:, bb * 128:(bb + 1) * 128],
                            lhsT=T2v[:, d2i[2][dlt], :], rhs=pooled1[:, s, :],
                            start=(i == 0), stop=(i == len(ovl) - 1))
                hi_rb = work.tile([64, 512], BF16, tag="hi2", name="hi2")
                nc.scalar.activation(hi_rb[:], ps[64:128, :], AF.Relu,
                                     bias=cf[0:64, CF_B2:CF_B2 + 1])

                def _alt(t, off):  # blocks (off, off+2) of a [64,512] view
                    s = t[0:64, off * 128:] if t.shape[0] == 128 else t[:, off * 128:]
                    return AP(s.tensor, s.offset, [list(s.ap[0]), [256, 2], [1, 128]])

                for par in range(2):  # block parity: 0 -> base 0, 1 -> base 64
                    nc.vector.scalar_tensor_tensor(
                        act3[64 * par:64 * par + 64, 2 * g:2 * g + 2, :],
                        in0=_alt(ps, par),
                        scalar=cf[0:64, CF_B2:CF_B2 + 1],
                        in1=_alt(hi_rb, par),
                        op0=OP.add, op1=OP.max)

            # embed: relu(h@We + eb) + pe == max(h@We, -eb) + (eb + pe)
            pse = psum.tile([128, D], F32, tag="psB", name="psB")
            for c in range(16):
                nc.tensor.matmul(pse[:], lhsT=act3[:, c, :], rhs=We[:, c, :],
                                 start=(c == 0), stop=(c == 15))
            er = work.tile([128, D], F32, tag="er", name="er")
            nc.vector.tensor_tensor(er[:], pse[:], cb[:, CB_NEB:CB_NEB + D], OP.max)
            nc.gpsimd.tensor_tensor(t_rm[rt][:], er[:], cf[:, CF_EBPE:CF_EBPE + D],
                                    OP.add)

        # ------------------------------------------------------- transformer
        for lyr in range(n_layers):
            _mark(nc, f'L{lyr}_wload')
            wq = wpool.tile([128, 3, 128 * (H // 2)], BF16, tag="wq", name="wq")
            wk = wpool.tile([128, 3, 128 * (H // 2)], BF16, tag="wk", name="wk")
            wv = wpool.tile([128, 3, D], BF16, tag="wv", name="wv")
            wo = wpool.tile([128, 3, D], BF16, tag="wo", name="wo")
            w1 = wpool.tile([128, 3, DFF], BF16, tag="w1", name="w1")
            w2 = wpool.tile([128, 12, D], BF16, tag="w2", name="w2")
            nc.sync.dma_start(wq[:], wq_d[lyr].rearrange("(c p) m -> p c m", c=3))
            nc.sync.dma_start(wk[:], wk_d[lyr].rearrange("(c p) m -> p c m", c=3))
            nc.sync.dma_start(wv[:], wv_d[lyr].rearrange("(c p) m -> p c m", c=3))
            nc.sync.dma_start(wo[:], wo_d[lyr].rearrange("(c p) m -> p c m", c=3))
            nc.sync.dma_start(w1[:], w1_d[lyr].rearrange("(c p) m -> p c m", c=3))
            nc.sync.dma_start(w2[:], w2_d[lyr].rearrange("(c p) m -> p c m", c=12))
            lsm = wpool.tile([128, 20], F32, tag="lsm", name="lsm")
            nc.sync.dma_start(lsm[:], lsmall_d[lyr])
            lb7 = wpool.tile([128, 7 * D], BF16, tag="lb7", name="lb7")
            nc.sync.dma_start(lb7[:], lb7_d[lyr])

            def lb(nm):
                return lb7[:, LB[nm]:LB[nm] + D]

            # t_fm <- transpose(t_rm)
            _mark(nc, f'L{lyr}_tfm')
            for rt in range(RPC):
                for c in range(3):
                    ps = psum.tile([128, 128], F32, tag="psC", name="psC")
                    nc.tensor.transpose(ps[:], t_rm[rt][:, c * 128:(c + 1) * 128], idn_f)
                    if (rt + c) % 2 == 0:
                        nc.scalar.copy(t_fm[c][:, rt * 128:(rt + 1) * 128], ps[:])
                    else:
                        nc.vector.tensor_copy(t_fm[c][:, rt * 128:(rt + 1) * 128], ps[:])

            # u = o @ Wo ; x1 = t + u + bo ; LN1 -> t_rm; then the FFN
            # transpose for that row-tile. Emitted per-sample inside the
            # attention loop so the LN chain overlaps neighboring samples.
            def wo_ln1_rt(rt):
                cs = slice(rt * 128, (rt + 1) * 128)
                pu = psum.tile([128, D], F32, tag="psB", name="psB")
                for c in range(3):
                    nc.tensor.matmul(pu[:], lhsT=o_fm[c][:, cs], rhs=wo[:, c, :],
                                     start=(c == 0), stop=(c == 2))
                x1 = work.tile([128, D], F32, tag="x1", name="x1")
                nc.vector.tensor_tensor(x1[:], pu[:], t_rm[rt][:], OP.add)
                (nc.gpsimd if rt % 2 else nc.vector).tensor_tensor(
                    x1[:], x1[:], lb("bo"), OP.add)
                layer_norm(rt, x1, LB["g1"], LB["be1"])
                for c in range(3):
                    ps = psum.tile([128, 128], F32, tag="psC", name="psC")
                    nc.tensor.transpose(ps[:], t_rm[rt][:, c * 128:(c + 1) * 128], idn_f)
                    if (rt + c) % 2 == 0:
                        nc.scalar.copy(t_fm[c][:, rt * 128:(rt + 1) * 128], ps[:])
                    else:
                        nc.vector.tensor_copy(t_fm[c][:, rt * 128:(rt + 1) * 128], ps[:])

            # Q, K batched over all samples; head pairs packed at partition
            # bases 0/64 within each 128-col chunk.
            _mark(nc, f'L{lyr}_attn')
            qf = state.tile([128, H // 2, R], BF16, tag="qf", name="qf")
            kf = state.tile([128, H // 2, R], BF16, tag="kf", name="kf")
            if phase >= 2:
                for dst, wmat, boff in ((qf, wq, 0), (kf, wk, 4)):
                    for p4 in range(H // 2):
                        for nh in range(2):
                            pq = psum.tile([128, 512], F32, tag="psD", name="psD")
                            for c in range(3):
                                nc.tensor.matmul(
                                    pq[:], lhsT=wmat[:, c, p4 * 128:(p4 + 1) * 128],
                                    rhs=t_fm[c][:, nh * 512:(nh + 1) * 512],
                                    start=(c == 0), stop=(c == 2))
                            dsl = dst[:, p4, nh * 512:(nh + 1) * 512]
                            nc.vector.tensor_scalar(dsl, pq[:],
                                                    lsm[:, boff + p4:boff + p4 + 1],
                                                    None, OP.add)
            for n in range(RPC) if phase >= 3 else []:
                cs = slice(n * 128, (n + 1) * 128)
                pv = psum.tile([128, D], F32, tag="psB", name="psB")
                for c in range(3):
                    nc.tensor.matmul(pv[:], lhsT=t_fm[c][:, cs], rhs=wv[:, c, :],
                                     start=(c == 0), stop=(c == 2))
                v_rm = work.tile([128, D], BF16, tag="v_rm", name="v_rm")
                nc.vector.tensor_tensor(v_rm[:], pv[:], lb("bv"), OP.add)
                if phase < 4:
                    continue

                # scores computed TRANSPOSED (S^T[s,q] per head) so the AV
                # matmul can read A^T straight from SBUF; softmax sums over
                # s become per-head colsum mask-matmuls accumulated on PE.
                # No max subtraction: logits are O(1)-bounded here, exp in
                # f32 psum is safe.
                es16 = work.tile([128, 8, 128], BF16, tag="es16", name="es16")
                rr = work.tile([128, 8], F32, tag="rr", name="rr")
                pssum = psum.tile([8, 128], F32, tag="psC", name="pssum")
                # heads grouped by pair-parity: each psum bank sees a single
                # PE tile_position base (mixing bases within one bank locks
                # up the device; across banks it is fine).
                pss_par = []
                for par in range(2):
                    pss = psum.tile([128, 512], F32,
                                    tag="psA" if par == 0 else "psD", name="pssc")
                    pss_par.append(pss)
                    for i, h in enumerate(range(par, H, 2)):
                        nc.tensor.matmul(
                            pss[:, i * 128:(i + 1) * 128],
                            lhsT=kf[64 * par:64 * par + 64, h // 2, cs],
                            rhs=qf[64 * par:64 * par + 64, h // 2, cs],
                            start=True, stop=True)
                if phase >= 5:
                    for j, h in enumerate([0, 2, 4, 6, 1, 3, 5, 7]):
                        pss = pss_par[h & 1]
                        nc.scalar.activation(
                            es16[:, h, :], pss[:, (h // 2) * 128:(h // 2 + 1) * 128],
                            AF.Exp, scale=TEMP)
                        nc.tensor.matmul(
                            pssum[:], lhsT=cb[:, CB_MASK + h * 8:CB_MASK + (h + 1) * 8],
                            rhs=es16[:, h, :], start=(j == 0), stop=(j == 7))
                if phase < 6:
                    continue
                ssb = work.tile([8, 128], F32, tag="ssb", name="ssb")
                nc.vector.tensor_copy(ssb[:], pssum[:])
                pst = psum.tile([128, 8], F32, tag="psD", name="psT")
                nc.tensor.matmul(pst[:], lhsT=ssb[:], rhs=idn_f[0:8, 0:8],
                                 start=True, stop=True)
                ssum = work.tile([128, 8], F32, tag="ssum", name="ssum")
                nc.vector.tensor_copy(ssum[:], pst[:])
                nc.vector.reciprocal(rr[:], ssum[:])
                if phase < 7:
                    continue

                pso = psum.tile([128, D], F32, tag="psB", name="psB")
                for h in range(H):
                    nc.tensor.matmul(pso[:, h * 48:(h + 1) * 48], lhsT=es16[:, h, :],
                                     rhs=v_rm[:, h * 48:(h + 1) * 48], start=True, stop=True)
                o_rm = work.tile([128, D], BF16, tag="o_rm", name="o_rm")
                rrb = AP(rr.tensor, rr.offset, [list(rr.ap[0]), [1, 8], [0, 48]])
                nc.vector.tensor_tensor(o_rm[:].rearrange("p (a b) -> p a b", a=8),
                                        pso[:].rearrange("p (a b) -> p a b", a=8),
                                        rrb, OP.mult)
                for c in range(3):
                    ps = psum.tile([128, 128], BF16, tag="psC", name="psC")
                    nc.tensor.transpose(ps[:], o_rm[:, c * 128:(c + 1) * 128], idn_b)
                    nc.vector.tensor_copy(o_fm[c][:, cs], ps[:])
                if phase >= 8:
                    wo_ln1_rt(n)

            # u = o @ Wo ; x1 = t + u + bo ; LN1 -> t_rm
            def layer_norm(rt, x1, goff, beoff):
                bnt = work.tile([128, 6], F32, tag="bnt", name="bnt")
                ag = work.tile([128, 2], F32, tag="ag", name="ag")
                sd = work.tile([128, 1], F32, tag="sd", name="sd")
                rstd = work.tile([128, 1], F32, tag="rstd", name="rstd")
                nc.vector.bn_stats(bnt[:], x1[:])
                nc.vector.bn_aggr(ag[:], bnt[:])
                nc.scalar.activation(sd[:], ag[:, 1:2], AF.Sqrt,
                                     bias=cf[:, CF_EPS:CF_EPS + 1])
                nc.vector.reciprocal(rstd[:], sd[:])
                xn = work.tile([128, D], F32, tag="xn", name="xn")
                nc.vector.tensor_scalar(xn[:], x1[:], ag[:, 0:1], rstd[:],
                                        OP.subtract, OP.mult)
                eng = nc.gpsimd if rt % 2 else nc.vector
                eng.tensor_tensor(xn[:], xn[:], lb7[:, goff:goff + D], OP.mult)
                eng.tensor_tensor(t_rm[rt][:], xn[:], lb7[:, beoff:beoff + D],
                                  OP.add)

            # FFN
            _mark(nc, f'L{lyr}_ffn')
            if phase < 9:
                continue
            for nh in range(2):
                for dc in range(12):
                    ph = psum.tile([128, 512], F32, tag="psA" if (dc + nh) % 2 == 0
                                   else "psD", name="psA")
                    for c in range(3):
                        nc.tensor.matmul(ph[:], lhsT=w1[:, c, dc * 128:(dc + 1) * 128],
                                         rhs=t_fm[c][:, nh * 512:(nh + 1) * 512],
                                         start=(c == 0), stop=(c == 2))
                    nc.scalar.activation(h1[dc][:, nh * 512:(nh + 1) * 512], ph[:],
                                         AF.Relu, bias=lsm[:, 8 + dc:9 + dc])
            for rt in range(RPC):
                cs = slice(rt * 128, (rt + 1) * 128)
                py = psum.tile([128, D], F32, tag="psB", name="psB")
                for dc in range(12):
                    nc.tensor.matmul(py[:], lhsT=h1[dc][:, cs], rhs=w2[:, dc, :],
                                     start=(dc == 0), stop=(dc == 11))
                x2 = work.tile([128, D], F32, tag="x1", name="x1")
                nc.vector.tensor_tensor(x2[:], py[:], t_rm[rt][:], OP.add)
                (nc.gpsimd if rt % 2 else nc.vector).tensor_tensor(
                    x2[:], x2[:], lb("b2f"), OP.add)
                layer_norm(rt, x2, LB["g2"], LB["be2"])

        if dbg_d is not None:
            for rt in range(RPC):
                nc.sync.dma_start(dbg_d[rt * 128:(rt + 1) * 128, :], t_rm[rt][:])

        # ------------------------------------------------------- head
        # y_n = sum_{c,p} (t_rm[n]^T @ ones/L)[p, c] * clsw[p, c] + clsb
        _mark(nc, 'head')
        pm = psum.tile([128, 24], F32, tag="psD", name="psH")
        for n in range(RPC):
            for c in range(3):
                nc.tensor.matmul(pm[:, n * 3 + c:n * 3 + c + 1],
                                 lhsT=t_rm[n][:, c * 128:(c + 1) * 128],
                                 rhs=cf[:, CF_ONES:CF_ONES + 1],
                                 start=True, stop=True)
        tm = work.tile([128, 24], F32, tag="tm", name="tm")
        _cw = cf[:, CF_CLSW:CF_CLSW + 3]
        clswb = AP(_cw.tensor, _cw.offset, [list(_cw.ap[0]), [0, 8], [1, 3]])
        nc.vector.tensor_tensor(tm[:].rearrange("p (n c) -> p n c", n=8),
                                pm[:].rearrange("p (n c) -> p n c", n=8),
                                clswb, OP.mult)
        red = work.tile([128, 8], F32, tag="red", name="red")
        nc.vector.tensor_reduce(red[:], tm[:].rearrange("p (n c) -> p n c", n=8),
                                axis=AX.X, op=OP.add)
        red2 = work.tile([128, 8], F32, tag="red2", name="red2")
        nc.gpsimd.partition_all_reduce(red2[:], red[:], channels=128,
                                       reduce_op=bass.bass_isa.ReduceOp.add)
        outsb = state.tile([1, RPC], F32, tag="outsb", name="outsb")
        nc.scalar.activation(outsb[:], red2[0:1, :], AF.Identity, bias=clsb[:])
        nc.sync.dma_start(y_d[:].rearrange("a b -> b a"), outsb[:])

    if do_compile:
        nc.compile()
    return nc


_PROG = {}


def _get_prog(debug=None, n_layers=NL, phase=99, loop_k=1):
    key = ("dbg" if debug else "plain", n_layers, phase, loop_k)
    if key not in _PROG:
        _PROG[key] = build_program(debug, n_layers=n_layers, phase=phase,
                                   loop_k=loop_k)
    return _PROG[key]


def _in_maps(inputs):
    shared = host_prep(inputs)
    x = np.asarray(inputs["x"], np.float32)  # (64, 128, 256)
    in_maps = []
    for c in range(NCORES):
        m = dict(shared)
        m["xc"] = np.ascontiguousarray(
            x[c * RPC:(c + 1) * RPC].reshape(R, W))
        in_maps.append(m)
    return in_maps


def kernel(**inputs):
    nc = _get_prog()
    res = run_bass_kernel_spmd(nc, _in_maps(inputs), core_ids=list(range(NCORES)))
    out = np.concatenate([res.results[c]["yc"] for c in range(NCORES)], axis=0)
    return out.astype(np.float32)


def _time_exec(nc, in_maps, reps=12):
    """Min per-call wall time of one bass_exec of `nc` (device-resident IO)."""
    import time
    import jax
    from jax.experimental.shard_map import shard_map
    from jax.sharding import Mesh, NamedSharding, PartitionSpec
    from concourse import bass2jax, mybir as mb

    bass2jax.install_neuronx_cc_hook()
    partition_name = nc.partition_id_tensor.name if nc.partition_id_tensor else None
    in_names, out_names, out_avals, zero_outs = [], [], [], []
    for alloc in nc.m.functions[0].allocations:
        if not isinstance(alloc, mb.MemoryLocationSet):
            continue
        name = alloc.memorylocations[0].name
        if alloc.kind == "ExternalInput":
            if name != partition_name:
                in_names.append(name)
        elif alloc.kind == "ExternalOutput":
            shape = tuple(alloc.tensor_shape)
            dtype = mb.dt.np(alloc.dtype)
            out_avals.append(jax.core.ShapedArray(shape, dtype))
            out_names.append(name)
            zero_outs.append(np.zeros(shape, dtype))
    n_params, n_outs = len(in_names), len(out_avals)
    all_in = list(in_names) + list(out_names)
    if partition_name is not None:
        all_in.append(partition_name)

    def _body(*args):
        ins = list(args[:n_params])
        outs = list(args[n_params:])
        operands = ins + outs
        if partition_name is not None:
            operands = operands + [bass2jax.partition_id_tensor()]
        return tuple(bass2jax._bass_exec_p.bind(
            *operands, out_avals=tuple(out_avals), in_names=tuple(all_in),
            out_names=tuple(out_names), lowering_input_output_aliases=(),
            sim_require_finite=True, sim_require_nnan=True, nc=nc))

    devices = jax.devices()[:NCORES]
    mesh = Mesh(np.asarray(devices), ("core",))
    shard = NamedSharding(mesh, PartitionSpec("core"))
    dev_in = [jax.device_put(
        np.concatenate([np.asarray(in_maps[c][nm]) for c in range(NCORES)], axis=0),
        shard) for nm in in_names]
    zsh = [jax.device_put(
        np.zeros((NCORES * z.shape[0], *z.shape[1:]), z.dtype), shard)
        for z in zero_outs]
    f = jax.jit(
        shard_map(_body, mesh=mesh,
                  in_specs=(PartitionSpec("core"),) * (n_params + n_outs),
                  out_specs=(PartitionSpec("core"),) * n_outs, check_rep=False),
        keep_unused=True)
    out = f(*dev_in, *zsh)
    jax.block_until_ready(out)      # warm compile
    ts = []
    for _ in range(reps):
        t0 = time.perf_counter()
        out = f(*dev_in, *zsh)
        jax.block_until_ready(out)
        ts.append(time.perf_counter() - t0)
    host_out = [np.asarray(o) for o in out]
    return min(ts), dict(zip(out_names, host_out))


LOOP_K = 33


def timed_run(inputs, loop_k=LOOP_K, rounds=3):
    """HW exec time of one forward pass, measured as marginal cost.

    Per-exec dispatch through this axon tunnel has a large fixed overhead
    (a 1-instruction program costs the same wall time as the full kernel)
    which flips between a fast and a slow state per executable load, so
    single-shot wall time says nothing about kernel speed.  Instead we
    build the same program with the whole body wrapped in a hardware For_i
    loop of `loop_k` iterations, measure each program over several
    interleaved executable loads to find its fast-state floor, and report
        (t(loop_k) - t(1)) / (loop_k - 1)
    which cancels the fixed dispatch overhead and the one-time constant
    preamble.  Also cross-checks that the looped program computes the same
    output.  Returns ns per forward pass.
    """
    in_maps = _in_maps(inputs)
    nc1, nck = _get_prog(), _get_prog(loop_k=loop_k)
    t1s, tks = [], []
    out1 = outk = None
    for _ in range(rounds):
        t1, out1 = _time_exec(nc1, in_maps, reps=8)
        tk, outk = _time_exec(nck, in_maps, reps=8)
        t1s.append(t1)
        tks.append(tk)
    y1, yk = out1["yc"], outk["yc"]
    if not np.allclose(y1, yk, atol=1e-5, rtol=1e-3):
        print(f"WARNING: loop_k output mismatch (max abs diff "
              f"{np.abs(y1 - yk).max():.3e})")
    return int((min(tks) - min(t1s)) / (loop_k - 1) * 1e9)


def debug_run(inputs, core=0, n_layers=NL, ncores=1, phase=99):
    """Run the debug program; returns (y, t_rm_dump) for one core."""
    nc = _get_prog(debug=True, n_layers=n_layers, phase=phase)
    res = run_bass_kernel_spmd(nc, _in_maps(inputs)[:ncores], core_ids=list(range(ncores)))
    return res.results[core]["yc"], res.results[core]["dbg"]
